# revision 28
# baseline (speedup 1.0000x reference)
"""Causal self-attention (B=4, T=2048, D=1024, H=16, hd=64) on 8 trn2 NeuronCores.

Sharding: data parallel over batch (4) x tensor parallel over heads (2 groups
of 8). Core c handles batch c//2 and heads (c%2)*8 .. (c%2)*8+8.
Wq/Wk/Wv are column-parallel by head group, Wo row-parallel; the pair of
cores sharing a batch produce partial outputs that are summed on the host.

On-device layout (per core) is fully "transposed": projections produce
Q^T, K^T [512, 2048] and V [2048, 512], scores are computed as
S^T = K Q^T (j=key on partitions, i=query on free dim), softmax uses
exp without max subtraction (scores are O(6) here), the denominator
comes for free from a ones-column appended to V, and attention output
O^T [hd, T] feeds the row-parallel out-projection directly as lhsT.

Schedule: one flat software-pipelined stream over (chunk, head-pair)
units, interleaved so exp-heavy later-chunk units pull forward into the
projection-heavy opening. Per j-tile the PE emits S (row-packed pair),
the exp of the previous j-tile goes to the scalar engine, and the AV of
the j-tile before that (lag 2, so AV never waits on its exp). All other
work — Q/K/V projections (split in 4-matmul halves) and the
out-projection — drains as due-slot-paced fillers between attention
slots. Exp and S are trimmed to the causally-valid query range of each
diagonal tile. Startup DMAs are contiguous (chunk-major x, mt-major
w) and spread across the three DGE queues; warmup matmuls ramp the PE
p-state during the DMA wait; output is written bf16 (partials summed
in f32 on the host).
"""

import contextlib
import ctypes
import sys
import types

import numpy as np

B, T, D = 4, 2048, 1024
H_TOT, HD = 16, 64
SCALE = HD ** -0.5
P = 128
NH = 8            # heads per core
QD = NH * HD      # 512, projected dim per core
KT = D // P       # 8 contraction tiles for projections
MT = QD // P      # 4 qdim tiles
TT = T // P       # 16 token tiles
ACH = 512         # token chunk; PSUM bank caps matmul N at 512
NACH = T // ACH   # 4
ICH = 512         # attention query chunk
NIC = T // ICH    # 4

_PROGRAM = None  # compiled program cache — build once per process


def _install_ntff_hook():
    """antenv.axon_hooks is missing in this image; recreate it so
    run_bass_kernel_spmd(trace=True) can profile. Harmless if unused."""
    if "antenv.axon_hooks" in sys.modules:
        return
    try:
        import antenv
    except ImportError:
        return
    mod = types.ModuleType("antenv.axon_hooks")
    _hook = [None]
    mod.set_axon_ntff_profile_hook = lambda h: _hook.__setitem__(0, h)
    mod.get_axon_ntff_profile_hook = lambda: _hook[0]
    antenv.axon_hooks = mod
    sys.modules["antenv.axon_hooks"] = mod
    try:
        lib = ctypes.CDLL("/opt/axon/libaxon_pjrt.so")
        if not hasattr(lib, "axon_start_nrt_profile"):
            return
        lib.axon_start_nrt_profile.argtypes = [
            ctypes.POINTER(ctypes.c_int64), ctypes.c_size_t]
        lib.axon_start_nrt_profile.restype = ctypes.c_int64
        lib.axon_stop_nrt_profile.argtypes = [ctypes.c_char_p]
        lib.axon_stop_nrt_profile.restype = ctypes.c_int64

        @contextlib.contextmanager
        def _hookfn(output_dir, device_ids):
            import jax
            jax.devices()
            if device_ids:
                ids = (ctypes.c_int64 * len(device_ids))(*device_ids)
                rc = lib.axon_start_nrt_profile(ids, len(device_ids))
            else:
                rc = lib.axon_start_nrt_profile(None, 0)
            if rc != 0:
                raise RuntimeError(f"axon_start_nrt_profile rc={rc}")
            try:
                yield
            finally:
                n = lib.axon_stop_nrt_profile(str(output_dir).encode())
                print(f"profile: {n} file(s) written to {output_dir}")

        mod.set_axon_ntff_profile_hook(_hookfn)
    except OSError:
        pass


def _build_program():
    from contextlib import ExitStack

    import concourse.tile as tile
    from concourse import bacc, mybir

    F32 = mybir.dt.float32
    BF16 = mybir.dt.bfloat16
    AF = mybir.ActivationFunctionType
    ALU = mybir.AluOpType

    nc = bacc.Bacc("TRN2", target_bir_lowering=False, debug=False,
                   num_devices=8)

    # all tensor inputs arrive pre-arranged in SBUF layout [128, k, n]
    # (host does the transpose) so every DMA is long contiguous runs
    xT_d = nc.dram_tensor("xT", [P, KT * T], BF16, kind="ExternalInput").ap()
    wq_d = nc.dram_tensor("wq", [P, KT * QD], BF16, kind="ExternalInput").ap()
    wk_d = nc.dram_tensor("wk", [P, KT * QD], BF16, kind="ExternalInput").ap()
    wv_d = nc.dram_tensor("wv", [P, KT * QD], BF16, kind="ExternalInput").ap()
    wo_d = nc.dram_tensor("wo", [P, MT * D], BF16, kind="ExternalInput").ap()
    bq_d = nc.dram_tensor("bq", [P, MT], F32, kind="ExternalInput").ap()
    bk_d = nc.dram_tensor("bk", [P, MT], F32, kind="ExternalInput").ap()
    bvb_d = nc.dram_tensor("bvb", [P, QD], F32, kind="ExternalInput").ap()
    msk_d = nc.dram_tensor("msk", [P, P], BF16, kind="ExternalInput").ap()
    out_d = nc.dram_tensor("out", [T, D], BF16, kind="ExternalOutput").ap()

    xT_c = xT_d.rearrange("p (c k t) -> p c k t", c=NACH, k=KT)
    # wq/wk arrive mt-major so the first head-pair's weights are one
    # contiguous run and can land first
    wq_m = wq_d.rearrange("p (m k q) -> p m k q", m=MT, k=KT)
    wk_m = wk_d.rearrange("p (m k q) -> p m k q", m=MT, k=KT)
    wv_k = wv_d.rearrange("p (k m) -> p k m", k=KT)
    wo_k = wo_d.rearrange("p (k e) -> p k e", k=MT)      # [128, 4, 1024]

    with tile.TileContext(nc) as tc, ExitStack() as ctx:
        persist = ctx.enter_context(tc.tile_pool(name="persist", bufs=1))

        qt = [persist.tile([P, T], BF16, name=f"qt{i}") for i in range(MT)]
        kt_ = [persist.tile([P, T], BF16, name=f"kt{i}") for i in range(MT)]
        v3 = [persist.tile([P, NH, HD + 1], BF16, name=f"v3_{i}")
              for i in range(TT)]
        at = [persist.tile([P, T], BF16, name=f"at{i}") for i in range(MT)]
        xt_all = persist.tile([P, NACH, KT, ACH], BF16, name="xt")

        wq_sb = persist.tile([P, MT, KT, P], BF16, name="wq")
        wk_sb = persist.tile([P, MT, KT, P], BF16, name="wk")
        bq_sb = persist.tile([P, MT], F32, name="bq")
        bk_sb = persist.tile([P, MT], F32, name="bk")
        bvb_sb = persist.tile([P, NH, HD], F32, name="bvb")
        tri_sb = persist.tile([P, P], BF16, name="tri")
        wv_sb = persist.tile([P, KT, QD], BF16, name="wv")
        wo_sb = persist.tile([P, MT, D], BF16, name="wo")

        # Startup DMAs spread over the three DMA-capable queues (sync/SP,
        # scalar/Activation, gpsimd; ~90 GB/s each) with deadline-aware
        # order: wq/wk head-pair 0 + the first x chunk (k-halves split
        # across two queues) land ~12us in; x chunk 1 by ~20us.
        nc.sync.dma_start(xt_all[:, 0, 0:4], xT_c[:, 0, 0:4])
        nc.sync.dma_start(xt_all[:, 1, 0:4], xT_c[:, 1, 0:4])
        nc.sync.dma_start(xt_all[:, 2, 0:4], xT_c[:, 2, 0:4])
        nc.sync.dma_start(xt_all[:, 3, 0:4], xT_c[:, 3, 0:4])
        nc.scalar.dma_start(xt_all[:, 0, 4:8], xT_c[:, 0, 4:8])
        nc.scalar.dma_start(wq_sb[:, 0], wq_m[:, 0])
        nc.scalar.dma_start(bq_sb[:], bq_d)
        nc.scalar.dma_start(xt_all[:, 1, 4:8], xT_c[:, 1, 4:8])
        nc.scalar.dma_start(wq_sb[:, 1], wq_m[:, 1])
        nc.scalar.dma_start(
            bvb_sb[:], bvb_d.rearrange("p (h d) -> p h d", d=HD))
        nc.scalar.dma_start(wq_sb[:, 2:4], wq_m[:, 2:4])
        nc.gpsimd.dma_start(tri_sb[:], msk_d)
        nc.gpsimd.dma_start(wk_sb[:, 0], wk_m[:, 0])
        nc.gpsimd.dma_start(wv_sb[:, 0:4], wv_k[:, 0:4])
        nc.gpsimd.dma_start(bk_sb[:], bk_d)
        nc.gpsimd.dma_start(wv_sb[:, 4:8], wv_k[:, 4:8])
        nc.gpsimd.dma_start(wk_sb[:, 1], wk_m[:, 1])
        nc.gpsimd.dma_start(xt_all[:, 2, 4:8], xT_c[:, 2, 4:8])
        nc.gpsimd.dma_start(wk_sb[:, 2:4], wk_m[:, 2:4])
        nc.gpsimd.dma_start(xt_all[:, 3, 4:8], xT_c[:, 3, 4:8])
        nc.gpsimd.dma_start(wo_sb[:], wo_k)
        ones64 = persist.tile([1, HD], BF16, name="ones64")
        nc.vector.memset(ones64[:], 1.0)
        for tt in range(TT):
            nc.vector.memset(v3[tt][:, :, HD:HD + 1], 1.0)

        # ---- runway: Q/K for (chunk 0, head-pair 0) in a short-lived pool
        # whose banks free up before the attention pools open ------------
        with tc.tile_pool(name="rway", bufs=1, space="PSUM") as rp:
            # warm up the PE p-state on the (tiny, early-landing) mask tile
            # while the first x/w DMAs are still in flight
            warm = rp.tile([P, P], F32, name="warm", bufs=1)
            for _ in range(48):
                nc.tensor.matmul(warm[:], tri_sb[:], tri_sb[:],
                                 start=True, stop=True)
            for eng, (w_sb, dst, b_sb) in (
                    (0, (wk_sb, kt_, bk_sb)), (1, (wq_sb, qt, bq_sb))):
                ps = rp.tile([P, ACH], F32, name="rw", bufs=2)
                for k in range(KT):
                    nc.tensor.matmul(ps[:], w_sb[:, 0, k, :],
                                     xt_all[:, 0, k, :],
                                     start=(k == 0), stop=(k == KT - 1))
                if eng == 0:
                    nc.scalar.add(dst[0][:, 0:ACH], ps[:], b_sb[:, 0:1])
                else:
                    nc.vector.tensor_scalar_add(dst[0][:, 0:ACH], ps[:],
                                                b_sb[:, 0:1])

        # ---- attention + fillers, one fused software-pipelined stream ----
        with tc.tile_pool(name="attnsb", bufs=1) as ap_, \
             tc.tile_pool(name="obp", bufs=3) as obp, \
             tc.tile_pool(name="attnps", bufs=1, space="PSUM") as sp:

            half_ps = {}

            def emit_projqk(c, hp, which, half, early=False):
                """Half a Q/K projection for (chunk c, hp): 4 k-steps per
                drain so a filler never delays the next S-pair enough to
                starve the exp stream."""
                w_sb, dst, b_sb = ((wq_sb, qt, bq_sb) if which == 0
                                   else (wk_sb, kt_, bk_sb))
                key = ("qk", c, hp, which)
                if half == 0:
                    half_ps[key] = sp.tile([P, ACH], F32, name="misc",
                                           bufs=2)
                ps = half_ps[key]
                csl = slice(c * ACH, (c + 1) * ACH)
                for k in range(4 * half, 4 * half + 4):
                    nc.tensor.matmul(ps[:], w_sb[:, hp, k, :],
                                     xt_all[:, c, k, :],
                                     start=(k == 0), stop=(k == KT - 1))
                if half == 1:
                    del half_ps[key]
                    if early:
                        # scalar engine is exp-starved in the opening;
                        # run the bias-add there instead of DVE
                        nc.scalar.add(dst[hp][:, csl], ps[:],
                                      b_sb[:, hp:hp + 1])
                    else:
                        nc.vector.tensor_scalar_add(dst[hp][:, csl], ps[:],
                                                    b_sb[:, hp:hp + 1])

            def emit_v_tile(tt, half):
                key = ("v", tt)
                if half == 0:
                    half_ps[key] = sp.tile([P, QD], F32, name="misc",
                                           bufs=2)
                psv = half_ps[key]
                to = (tt % 4) * P
                for k in range(4 * half, 4 * half + 4):
                    nc.tensor.matmul(
                        psv[:], xt_all[:, tt // 4, k, to:to + P],
                        wv_sb[:, k, :], start=(k == 0), stop=(k == KT - 1))
                if half == 1:
                    del half_ps[key]
                    nc.vector.tensor_tensor(
                        v3[tt][:, :, 0:HD],
                        psv[:].rearrange("p (h d) -> p h d", d=HD),
                        bvb_sb[:], op=ALU.add)

            def emit_out_group(mt, nch2, alt=False):
                if alt:
                    # tail-only: borrow an (idle by then) spsum-tag slot so
                    # consecutive groups double-buffer instead of serializing
                    pso = sp.tile([P, 2 * ICH], F32, name="spsum",
                                  bufs=2)[:, 0:512]
                else:
                    pso = sp.tile([P, 512], F32, name="misc", bufs=2)
                for k in range(MT):
                    nc.tensor.matmul(
                        pso[:], at[k][:, mt * P:(mt + 1) * P],
                        wo_sb[:, k, nch2 * 512:(nch2 + 1) * 512],
                        start=(k == 0), stop=(k == MT - 1))
                ob = obp.tile([P, 512], BF16, name="ob")
                nc.vector.tensor_copy(ob[:], pso[:])
                eng = (nc.sync, nc.scalar, nc.gpsimd)[(2 * mt + nch2) % 3]
                eng.dma_start(
                    out_d[mt * P:(mt + 1) * P,
                          nch2 * 512:(nch2 + 1) * 512], ob[:])

            # ---- flattened attention pipeline over interleaved units ----
            # Units are (chunk, head-pair) blocks ordered so exp-heavy
            # later-chunk units pull forward into the projection-heavy
            # opening, keeping the scalar engine fed. The j-tile stream is
            # software-pipelined S one ahead, globally, across unit
            # boundaries.
            units = [(0, 0), (1, 0), (0, 1), (1, 1), (2, 0), (0, 2),
                     (1, 2), (2, 1), (3, 0), (0, 3), (1, 3), (2, 2),
                     (3, 1), (2, 3), (3, 2), (3, 3)]
            stream = [(ic, hp, jt) for (ic, hp) in units
                      for jt in range(4 * ic + 4)]

            s2s, e2s, opst = {}, {}, {}
            pending = []

            def emit_s(ic, hp, jt):
                s2 = sp.tile([P, 2 * ICH], F32, name="spsum", bufs=2)
                jsl = slice(jt * P, (jt + 1) * P)
                c0 = max(jt - 4 * ic, 0) * P
                qsl = slice(ic * ICH + c0, (ic + 1) * ICH)
                nc.tensor.matmul(s2[:, c0:ICH], kt_[hp][0:HD, jsl],
                                 qt[hp][0:HD, qsl], start=True, stop=True)
                nc.tensor.matmul(s2[:, ICH + c0:2 * ICH],
                                 kt_[hp][HD:P, jsl],
                                 qt[hp][HD:P, qsl], start=True, stop=True)
                s2s[(ic, hp, jt)] = s2

            def emit_exp(ic, hp, jt):
                e2 = ap_.tile([P, 2 * ICH], BF16, name="e", bufs=4)
                s2 = s2s.pop((ic, hp, jt))
                kdiag = jt - 4 * ic
                c0 = max(kdiag, 0) * P
                if kdiag == 3:
                    # two small valid ranges; split beats one span
                    nc.scalar.activation(e2[:, c0:ICH], s2[:, c0:ICH],
                                         AF.Exp)
                    nc.scalar.activation(e2[:, ICH + c0:2 * ICH],
                                         s2[:, ICH + c0:2 * ICH], AF.Exp)
                else:
                    # single span from first valid col of head A to the
                    # end; covers head B's dead cols but one ACT's fixed
                    # overhead beats two for small c0
                    nc.scalar.activation(e2[:, c0:2 * ICH],
                                         s2[:, c0:2 * ICH], AF.Exp)
                if kdiag >= 0:
                    # zero the diagonal block's upper triangle
                    for half in range(2):
                        o = half * ICH + c0
                        nc.vector.tensor_tensor(
                            e2[:, o:o + P], e2[:, o:o + P],
                            tri_sb[:], op=ALU.mult)
                e2s[(ic, hp, jt)] = e2

            def emit_av(ic, hp, jt):
                need(("V", jt))
                njt = 4 * ic + 4
                if jt == 0:
                    opst[(ic, hp)] = (
                        sp.tile([HD + 1, ICH], F32, name="opsum", bufs=2),
                        sp.tile([HD + 1, ICH], F32, name="opsum", bufs=2))
                opsA, opsB = opst[(ic, hp)]
                kdiag = jt - 4 * ic
                c0 = max(kdiag, 0) * P
                e2 = e2s.pop((ic, hp, jt))
                nc.tensor.matmul(opsA[:, c0:], v3[jt][:, 2 * hp, :],
                                 e2[:, c0:ICH],
                                 start=(jt == 0), stop=(jt == njt - 1))
                nc.tensor.matmul(opsB[:, c0:], v3[jt][:, 2 * hp + 1, :],
                                 e2[:, ICH + c0:2 * ICH],
                                 start=(jt == 0), stop=(jt == njt - 1))
                if jt == njt - 1:
                    isl = slice(ic * ICH, (ic + 1) * ICH)
                    opsA, opsB = opst.pop((ic, hp))
                    last = (ic, hp) == (3, 3)

                    def normalize(hp=hp, isl=isl, opsA=opsA, opsB=opsB,
                                  last=last):
                        # first copy both accumulators (incl. the ones-row
                        # denominators) out of PSUM so the banks free for
                        # the next unit's AV; the rest of the chain runs
                        # from SBUF
                        ots = []
                        for ops in (opsA, opsB):
                            ot = ap_.tile([HD + 1, ICH], F32, name="ot",
                                          bufs=4)
                            nc.vector.tensor_copy(ot[:], ops[:])
                            ots.append(ot)
                        for half, ot in enumerate(ots):
                            po = half * HD
                            dn = ap_.tile([1, ICH], F32, name="dn", bufs=4)
                            nc.vector.tensor_copy(dn[:], ot[HD:HD + 1, :])
                            recip = ap_.tile([1, ICH], F32, name="recip",
                                             bufs=4)
                            nc.vector.reciprocal_approx_fast(recip[:], dn[:])
                            if last:
                                # tail-only: broadcast 1/d via a rank-1 PE
                                # matmul into an (idle by now) spsum bank —
                                # the serial gpsimd broadcasts would hold
                                # up the final out-projection ~2.5us
                                rbf = ap_.tile([1, ICH], BF16, name="rbf",
                                               bufs=2)
                                nc.vector.tensor_copy(rbf[:], recip[:])
                                rbp = sp.tile([P, 2 * ICH], F32,
                                              name="spsum", bufs=2)
                                nc.tensor.matmul(rbp[0:HD, 0:ICH],
                                                 ones64[:], rbf[:],
                                                 start=True, stop=True)
                                nc.vector.tensor_tensor(
                                    at[hp][po:po + HD, isl],
                                    rbp[0:HD, 0:ICH], ot[0:HD, :],
                                    op=ALU.mult)
                            else:
                                rb = ap_.tile([HD, ICH], F32, name="rb",
                                              bufs=4)
                                nc.gpsimd.partition_broadcast(rb[:],
                                                              recip[:])
                                nc.vector.tensor_tensor(
                                    at[hp][po:po + HD, isl], ot[0:HD, :],
                                    rb[:], op=ALU.mult)

                    pending.append(normalize)

            # ---- global filler schedule: (due_slot, tag, fn) ------------
            def FQK(c, hp, which, half, early=False):
                tag = ("QK", c, hp, which) if half == 1 else None
                return (tag,
                        lambda: emit_projqk(c, hp, which, half, early))

            def FV(tt, half):
                tag = ("V", tt) if half == 1 else None
                return (tag, lambda: emit_v_tile(tt, half))

            def FO(mt, n):
                return (None, lambda: emit_out_group(mt, n))

            sched = []
            for args in [
                (0, "v", 0, 0), (0, "v", 0, 1), (1, "v", 1, 0),
                (1, "v", 1, 1),
                (0, "q", 1, 0, 0, 0), (1, "q", 1, 0, 0, 1),
                (2, "q", 1, 0, 1, 0), (3, "q", 1, 0, 1, 1),
                (2, "v", 2, 0), (3, "v", 2, 1),
                (4, "v", 3, 0), (4, "v", 3, 1),
                (5, "q", 0, 1, 0, 0), (6, "v", 4, 0),
                (7, "q", 0, 1, 0, 1), (7, "v", 4, 1),
                (8, "v", 5, 0), (8, "v", 5, 1),
                (9, "q", 0, 1, 1, 0), (9, "v", 6, 0),
                (10, "v", 6, 1), (10, "q", 0, 1, 1, 1),
                (11, "v", 7, 0), (11, "v", 7, 1),
                (12, "q", 1, 1, 0, 0), (13, "q", 1, 1, 0, 1),
                (14, "q", 1, 1, 1, 0), (15, "q", 1, 1, 1, 1),
                (17, "q", 2, 0, 0, 0), (19, "q", 2, 0, 0, 1),
                (21, "q", 2, 0, 1, 0), (22, "q", 2, 0, 1, 1),
                (25, "q", 0, 2, 0, 0), (27, "q", 0, 2, 0, 1),
                (28, "v", 8, 0), (29, "v", 8, 1),
                (29, "q", 0, 2, 1, 0), (30, "v", 9, 0),
                (31, "q", 0, 2, 1, 1), (31, "v", 9, 1),
                (32, "v", 10, 0), (32, "v", 10, 1),
                (33, "v", 11, 0), (33, "v", 11, 1),
                (34, "q", 1, 2, 0, 0), (35, "q", 1, 2, 0, 1),
                (37, "q", 1, 2, 1, 0), (38, "q", 1, 2, 1, 1),
                (40, "q", 2, 1, 0, 0), (42, "q", 2, 1, 0, 1),
                (44, "q", 2, 1, 1, 0), (46, "q", 2, 1, 1, 1),
                (50, "q", 3, 0, 0, 0), (53, "q", 3, 0, 0, 1),
                (55, "q", 3, 0, 1, 0), (57, "q", 3, 0, 1, 1),
                (62, "q", 0, 3, 0, 0), (65, "q", 0, 3, 0, 1),
                (67, "v", 12, 0), (68, "v", 12, 1),
                (68, "q", 0, 3, 1, 0), (69, "v", 13, 0),
                (70, "v", 13, 1), (71, "q", 0, 3, 1, 1),
                (71, "v", 14, 0), (72, "v", 14, 1),
                (72, "v", 15, 0), (73, "v", 15, 1),
                (74, "q", 1, 3, 0, 0), (76, "q", 1, 3, 0, 1),
                (77, "q", 1, 3, 1, 0), (78, "q", 1, 3, 1, 1),
                (81, "q", 2, 2, 0, 0), (83, "q", 2, 2, 0, 1),
                (85, "q", 2, 2, 1, 0), (86, "q", 2, 2, 1, 1),
                (84, "o", 0, 0), (87, "o", 0, 1),
                (90, "q", 3, 1, 0, 0), (90, "o", 1, 0),
                (93, "q", 3, 1, 0, 1), (93, "o", 1, 1),
                (95, "q", 3, 1, 1, 0), (96, "o", 2, 0),
                (97, "q", 3, 1, 1, 1), (99, "o", 2, 1),
                (102, "o", 3, 0), (105, "o", 3, 1),
                (104, "q", 2, 3, 0, 0), (107, "q", 2, 3, 0, 1),
                (108, "o", 4, 0), (110, "q", 2, 3, 1, 0),
                (111, "o", 4, 1), (113, "q", 2, 3, 1, 1),
                (114, "o", 5, 0), (117, "o", 5, 1),
                (118, "q", 3, 2, 0, 0), (120, "q", 3, 2, 0, 1),
                (120, "o", 6, 0), (122, "q", 3, 2, 1, 0),
                (123, "o", 6, 1), (124, "q", 3, 2, 1, 1),
                (126, "o", 7, 0), (129, "o", 7, 1),
                (132, "o", 8, 0), (134, "q", 3, 3, 0, 0),
                (135, "o", 8, 1), (136, "q", 3, 3, 0, 1),
                (138, "o", 9, 0), (138, "q", 3, 3, 1, 0),
                (140, "q", 3, 3, 1, 1), (166, "o", 9, 1),
                (167, "o", 10, 0), (163, "o", 10, 1),
                (164, "o", 11, 0), (165, "o", 11, 1),
            ]:
                due, kind = args[0], args[1]
                if kind == "q":
                    c, hp, which, half = args[2:]
                    sched.append((due, FQK(c, hp, which, half,
                                           early=(due < 16))))
                elif kind == "v":
                    tt, half = args[2:]
                    sched.append((due, FV(tt, half)))
                else:
                    mt, n2 = args[2:]
                    sched.append((due, FO(mt, n2)))
            sched.sort(key=lambda x: x[0])
            dues = [d for d, _ in sched]
            fillers = [f for _, f in sched]
            tag_idx = {tag: i for i, (tag, _) in enumerate(fillers)
                       if tag is not None}
            drained = [0]

            def drain_to(i):
                while drained[0] <= i:
                    fillers[drained[0]][1]()
                    drained[0] += 1

            def need(tag):
                if tag in tag_idx:
                    drain_to(tag_idx[tag])

            def maybe_fill(slot):
                while (drained[0] < len(fillers)
                       and dues[drained[0]] <= slot):
                    fillers[drained[0]][1]()
                    drained[0] += 1

            # ---- run the stream -------------------------------------
            for idx, (ic, hp, jt) in enumerate(stream):
                if jt == 0:
                    need(("QK", ic, hp, 0))
                    need(("QK", ic, hp, 1))
                emit_s(ic, hp, jt)
                if idx >= 1:
                    emit_exp(*stream[idx - 1])
                if idx >= 2:
                    pic, php, pjt = stream[idx - 2]
                    emit_av(pic, php, pjt)
                    if pjt == 4 * pic + 3 and pending:
                        pending.pop(0)()
                maybe_fill(idx)
            emit_exp(*stream[-1])
            emit_av(*stream[-2])
            # flush leftover fillers BEFORE the last unit's normalize: the
            # at-tile dependency tracking is coarse, so an out-group emitted
            # after a later normalize would falsely wait on it
            drain_to(len(fillers) - 1)
            emit_av(*stream[-1])
            while pending:
                pending.pop(0)()
            for i, (mt, n) in enumerate(
                    (mt, n) for mt in range(12, 16) for n in range(2)):
                emit_out_group(mt, n, alt=(i % 2 == 1))

    nc.compile()
    return nc


def _get_program():
    global _PROGRAM
    if _PROGRAM is None:
        _install_ntff_hook()
        _PROGRAM = _build_program()
    return _PROGRAM


def _make_masks():
    """Multiplicative upper-triangle zero mask [128, 128] for the diagonal
    128x128 block of each S^T tile: entry (j, i) = 1 if j <= i else 0."""
    j = np.arange(P)[:, None]
    i = np.arange(P)[None, :]
    return (j <= i).astype(np.float32)


def make_in_maps(x, Wq, bq, Wk, bk, Wv, bv, Wo, bo):
    import ml_dtypes
    bf16 = ml_dtypes.bfloat16

    def sbl(a, k):
        """[k*128, n] -> SBUF layout [128, k*n] (partition-major runs)."""
        n = a.shape[1]
        return np.ascontiguousarray(
            a.reshape(k, P, n).transpose(1, 0, 2).reshape(P, k * n)
        ).astype(bf16)

    def sbl_mt(a):
        """[1024, 512] weight -> mt-major SBUF layout [128, MT*KT*128]."""
        return np.ascontiguousarray(
            a.reshape(KT, P, MT, P).transpose(1, 2, 0, 3).reshape(P, -1)
        ).astype(bf16)

    masks = _make_masks()
    in_maps = []
    for c in range(8):
        b, hg = c // 2, c % 2
        sl = slice(hg * QD, (hg + 1) * QD)
        in_maps.append({
            "xT": np.ascontiguousarray(
                np.asarray(x[b].T).reshape(KT, P, NACH, ACH)
                .transpose(1, 2, 0, 3).reshape(P, KT * T)).astype(bf16),
            "wq": sbl_mt(Wq[:, sl] * SCALE),
            "wk": sbl_mt(Wk[:, sl]),
            "wv": sbl(Wv[:, sl], KT),
            "wo": sbl(Wo[sl, :], MT),
            "bq": np.ascontiguousarray((bq[sl] * SCALE).reshape(MT, P).T),
            "bk": np.ascontiguousarray(bk[sl].reshape(MT, P).T),
            "bvb": np.ascontiguousarray(
                np.broadcast_to(bv[sl].astype(np.float32), (P, QD))),
            "msk": masks.astype(bf16),
        })
    return in_maps


def run(inputs, trace=False):
    from concourse.bass_utils import run_bass_kernel_spmd

    nc = _get_program()
    in_maps = make_in_maps(**inputs)
    res = run_bass_kernel_spmd(nc, in_maps, list(range(8)), trace=trace)
    bo = inputs["bo"]
    out = np.empty((B, T, D), dtype=np.float32)
    for b in range(B):
        out[b] = (res.results[2 * b]["out"].astype(np.float32)
                  + res.results[2 * b + 1]["out"].astype(np.float32) + bo)
    return out, res


def kernel(**inputs):
    inputs = {k: np.asarray(v) for k, v in inputs.items()}
    out, _ = run(inputs)
    return out
